# revision 7
# baseline (speedup 1.0000x reference)
"""Trainium2 Bass kernel for nn_NeuronCircuit_21157008900535 (moe_routing).

Data-parallel over batch B=8 across 8 NeuronCores (one batch element per core).
Per core, everything is computed in a transposed (feature-major) layout so no
on-device transposes of activations are ever needed:

  Wc[d,r]   = sum_k w_k * selc[k,d,r]          (PE, scaled-identity accumulate)
  Wq/k/v    = sum_k w_k * sele[k,r,d]          (PE, scaled-identity accumulate)
  hT[r,s]   = sum_dt Wc[dt]^T xT[dt]           (PE, f32r)
  QT/KT     = W*^T hT  per head  [64, S]       (PE, f32r)
  V[s,d']   = hT^T Wv  (natural) + ones column (PE, f32r -> bf16)
  stats     : S_nat = QT^T KT, causal row-max m[q] (DVE reduce, negate)
              -m written into QTe row 64 via PE-transpose + tiny DMA
  ST'[k,q]  = [KT;1]^T [QT;-m] = S - m[q]      (PE, f32r, K=65)
  E         = exp(0.125*ST') with causal mask  (ACT -> bf16)
  O'[dh+1,q]= [V|1]^T E   (AV + denominator in one matmul, bf16)
  OT        = O'[0:64] ; denom = O'[64]
  OT       /= denom  (recip + partition-broadcast DMA + DVE mul)
  Y[s,j]    = sum_dt OT[dt]^T W_OT[dt]         (PE, f32r)
"""
import numpy as np

import concourse.bacc as bacc
import concourse.mybir as mybir
import concourse.tile as tile
from concourse.masks import make_identity

B, S, D = 8, 1024, 1024
H, DH = 16, 64
R = 128
KC, KE = 16, 8
F32 = mybir.dt.float32
F32R = mybir.dt.float32r
BF16 = mybir.dt.bfloat16
NEG = -1.0e9


def build_program():
    nc = bacc.Bacc("TRN2", target_bir_lowering=False, debug=False)

    xt_d = nc.dram_tensor("xt", [D, S], F32R, kind="ExternalInput")
    selc_d = nc.dram_tensor("selc", [KC, D, R], F32R, kind="ExternalInput")
    sele_d = nc.dram_tensor("sele", [3 * KE, R, D], F32R, kind="ExternalInput")
    wvec_d = nc.dram_tensor("wvec", [1, 64], F32, kind="ExternalInput")
    wot_d = nc.dram_tensor("wot", [D, D], F32R, kind="ExternalInput")
    y_d = nc.dram_tensor("y", [S, D], F32, kind="ExternalOutput")
    recip_d = nc.dram_tensor("recip_scratch", [16, S], F32)

    NW = KC + 3 * KE  # 40 scaled identities

    with tile.TileContext(nc) as tc:
        with (
            tc.tile_pool(name="const", bufs=1) as const,
            tc.tile_pool(name="big", bufs=1) as big,
            tc.tile_pool(name="gather", bufs=3) as gather,
            tc.tile_pool(name="wotp", bufs=1) as wotp,
            tc.tile_pool(name="head", bufs=2) as headp,
            tc.tile_pool(name="small", bufs=3) as small,
            tc.tile_pool(name="ps", bufs=1, space="PSUM") as ps,  # 8 tags P0..P7
        ):
            _pn = [0]
            def P(i, shape=(128, 512), dtype=F32):
                _pn[0] += 1
                return ps.tile(list(shape), dtype, tag=f"P{i}", name=f"ps{i}_{_pn[0]}")

            # ---------------- constants ----------------
            id_sb = const.tile([128, 128], F32, tag="id")
            make_identity(nc, id_sb[:])
            # natural diag mask: maskn[q,k] = 0 if k<=q else NEG
            maskn = const.tile([128, 128], F32, tag="maskn")
            nc.gpsimd.memset(maskn[:], 0.0)
            nc.gpsimd.affine_select(
                out=maskn[:], in_=maskn[:],
                compare_op=mybir.AluOpType.is_ge, fill=NEG,
                base=0, pattern=[[-1, 128]], channel_multiplier=1,
            )
            # transposed diag mask: maskt[k,q] = 0 if k<=q else NEG
            maskt = const.tile([128, 128], F32, tag="maskt")
            nc.gpsimd.memset(maskt[:], 0.0)
            nc.gpsimd.affine_select(
                out=maskt[:], in_=maskt[:],
                compare_op=mybir.AluOpType.is_ge, fill=NEG,
                base=0, pattern=[[1, 128]], channel_multiplier=-1,
            )
            w_sb = const.tile([128, 64], F32, tag="w")
            nc.gpsimd.dma_start(out=w_sb[:], in_=wvec_d[0:1, :].to_broadcast((128, 64)))
            # scaled identities (f32r), one per selected neuron
            wI = const.tile([128, NW, 128], F32R, tag="wI")
            for k in range(NW):
                nc.vector.tensor_scalar_mul(wI[:, k, :], id_sb[:], w_sb[:, k : k + 1])

            # ---------------- big persistent tiles ----------------
            xt_sb = big.tile([128, 8, S], F32R, tag="xt")  # 32KB/part
            nc.sync.dma_start(out=xt_sb[:], in_=xt_d[:].rearrange("(t p) s -> p t s", p=128))
            wc_sb = big.tile([128, 8, 128], F32R, tag="wc")       # Wc[d-tile][d,r]
            we_sb = big.tile([128, 3, D], F32R, tag="we")         # Wq/Wk/Wv [r,d]
            ht_sb = big.tile([128, S], F32R, tag="ht")            # hT[r,s]
            ot_sb = [big.tile([128, S], F32R, tag=f"ot{dt}", name=f"ot{dt}") for dt in range(8)]
            vext = [big.tile([128, H, DH + 1], BF16, tag=f"vx{st}", name=f"vx{st}") for st in range(8)]
            denom = big.tile([16, S], F32, tag="den")
            recip = big.tile([16, S], F32, tag="rec")

            # ---------------- compress combine: Wc ----------------
            pc = [P(g, (128, 4, 128)) for g in range(2)]
            for n in range(KC):
                sc_n = gather.tile([128, 8, 128], F32R, tag="g")
                nc.sync.dma_start(out=sc_n[:], in_=selc_d[n].rearrange("(t p) r -> p t r", p=128))
                for g in range(2):
                    nc.tensor.matmul(
                        pc[g][:], wI[:, n, :], sc_n[:, g * 4 : (g + 1) * 4, :],
                        start=(n == 0), stop=(n == KC - 1),
                    )
            for g in range(2):
                nc.vector.tensor_copy(wc_sb[:, g * 4 : (g + 1) * 4, :], pc[g][:])

            # ---------------- expand combine: Wq, Wk, Wv ----------------
            pe = [P(2 + i) for i in range(6)]
            for p in range(3):
                for k in range(KE):
                    n = p * KE + k
                    se_n = gather.tile([128, D], F32R, tag="g")
                    nc.sync.dma_start(out=se_n[:], in_=sele_d[n])
                    for c in range(2):
                        nc.tensor.matmul(
                            pe[p * 2 + c][:], wI[:, KC + n, :], se_n[:, c * 512 : (c + 1) * 512],
                            start=(k == 0), stop=(k == KE - 1),
                        )
            for p in range(3):
                for c in range(2):
                    nc.vector.tensor_copy(we_sb[:, p, c * 512 : (c + 1) * 512], pe[p * 2 + c][:])

            # ---------------- hT = Wc^T @ xT ----------------
            for sc in range(2):
                ph = P(sc)
                for dt in range(8):
                    nc.tensor.matmul(
                        ph[:], wc_sb[:, dt, :], xt_sb[:, dt, sc * 512 : (sc + 1) * 512],
                        start=(dt == 0), stop=(dt == 7),
                    )
                nc.vector.tensor_copy(ht_sb[:, sc * 512 : (sc + 1) * 512], ph[:])

            # ---------------- V (natural) + ones column ----------------
            for st in range(8):
                nc.vector.memset(vext[st][:], 1.0)  # ones col; V copies overwrite cols 0:63
                for c in range(2):
                    pv = P(2 + (st % 2) * 2 + c)
                    nc.tensor.matmul(
                        pv[:], ht_sb[:, st * 128 : (st + 1) * 128],
                        we_sb[:, 2, c * 512 : (c + 1) * 512],
                        start=True, stop=True,
                    )
                    nc.vector.tensor_copy(
                        vext[st][:, c * 8 : (c + 1) * 8, 0:DH],
                        pv[:].rearrange("p (h e) -> p h e", e=DH),
                    )

            # ---------------- per-head SDPA ----------------
            for h in range(H):
                qte = headp.tile([65, S], F32R, tag="qte")
                kte = headp.tile([65, S], F32R, tag="kte")
                nc.vector.memset(kte[64:65, :].bitcast(F32), 1.0)
                for sc in range(2):
                    pq = P(0)
                    nc.tensor.matmul(
                        pq[0:64, :], we_sb[:, 0, h * 64 : (h + 1) * 64],
                        ht_sb[:, sc * 512 : (sc + 1) * 512], start=True, stop=True,
                    )
                    nc.vector.tensor_copy(qte[0:64, sc * 512 : (sc + 1) * 512], pq[0:64, :])
                    pk = P(1)
                    nc.tensor.matmul(
                        pk[0:64, :], we_sb[:, 1, h * 64 : (h + 1) * 64],
                        ht_sb[:, sc * 512 : (sc + 1) * 512], start=True, stop=True,
                    )
                    nc.vector.tensor_copy(kte[0:64, sc * 512 : (sc + 1) * 512], pk[0:64, :])

                # --- stats: causal row max per q-tile (negated) ---
                nmax = small.tile([128, 8], F32, tag="nmax")
                tmp2 = small.tile([128, 2], F32, tag="tmp2")
                for qi in range(8):
                    span = (qi + 1) * 128
                    c0 = min(span, 512)
                    pa = P(6)
                    nc.tensor.matmul(
                        pa[:, 0:c0], qte[0:64, qi * 128 : (qi + 1) * 128],
                        kte[0:64, 0:c0], start=True, stop=True,
                    )
                    if span > 512:
                        pb = P(7)
                        nc.tensor.matmul(
                            pb[:, 0 : span - 512], qte[0:64, qi * 128 : (qi + 1) * 128],
                            kte[0:64, 512:span], start=True, stop=True,
                        )
                        # mask diag block (lives in chunk 2)
                        off = qi * 128 - 512
                        nc.vector.tensor_add(pb[:, off : off + 128], pb[:, off : off + 128], maskn[:])
                        nc.vector.tensor_reduce(
                            out=tmp2[:, 0:1], in_=pa[:, 0:512],
                            axis=mybir.AxisListType.X, op=mybir.AluOpType.max,
                        )
                        nc.vector.tensor_reduce(
                            out=tmp2[:, 1:2], in_=pb[:, 0 : span - 512],
                            axis=mybir.AxisListType.X, op=mybir.AluOpType.max,
                        )
                        nc.vector.tensor_reduce(
                            out=nmax[:, qi : qi + 1], in_=tmp2[:, 0:2],
                            axis=mybir.AxisListType.X, op=mybir.AluOpType.max, negate=True,
                        )
                    else:
                        off = qi * 128
                        nc.vector.tensor_add(pa[:, off : off + 128], pa[:, off : off + 128], maskn[:])
                        nc.vector.tensor_reduce(
                            out=nmax[:, qi : qi + 1], in_=pa[:, 0:c0],
                            axis=mybir.AxisListType.X, op=mybir.AluOpType.max, negate=True,
                        )
                # transpose nmax -> row 64 of qte
                ptr = P(6)
                nc.tensor.transpose(ptr[0:8, 0:128], nmax[:], id_sb[:])
                trs = small.tile([8, 128], F32R, tag="trs")
                nc.vector.tensor_copy(trs[:], ptr[0:8, 0:128])
                nc.sync.dma_start(
                    out=qte[64:65, :].rearrange("p (a b) -> p a b", a=8), in_=trs[:]
                )

                # --- ST' + exp + AV ---
                for qc in range(2):
                    po = P(3 + qc)
                    nki = 4 * (qc + 1)
                    for ki in range(nki):
                        pst2 = P(1 + (ki % 2))
                        nc.tensor.matmul(
                            pst2[:], kte[:, ki * 128 : (ki + 1) * 128],
                            qte[:, qc * 512 : (qc + 1) * 512], start=True, stop=True,
                        )
                        e_t = small.tile([128, 512], BF16, tag="e")
                        off = ki * 128 - qc * 512  # diag block offset in this chunk
                        if off >= 0:
                            if off > 0:
                                nc.vector.memset(e_t[:, 0:off], 0.0)
                            nc.vector.tensor_add(
                                pst2[:, off : off + 128], pst2[:, off : off + 128], maskt[:]
                            )
                            nc.scalar.activation(
                                out=e_t[:, off:512], in_=pst2[:, off:512],
                                func=mybir.ActivationFunctionType.Exp, scale=0.125,
                            )
                        else:
                            nc.scalar.activation(
                                out=e_t[:], in_=pst2[:],
                                func=mybir.ActivationFunctionType.Exp, scale=0.125,
                            )
                        nc.tensor.matmul(
                            po[0:65, :], vext[ki][:, h, :], e_t[:],
                            start=(ki == 0), stop=(ki == nki - 1),
                        )
                    # OT rows and denominator row
                    nc.vector.tensor_copy(
                        ot_sb[h // 2][(h % 2) * 64 : (h % 2) * 64 + 64,
                                      qc * 512 : (qc + 1) * 512],
                        po[0:64, :],
                    )
                    dstage = small.tile([1, 512], F32, tag="dst")
                    nc.vector.tensor_copy(dstage[:], po[64:65, :])
                    nc.sync.dma_start(
                        out=denom[h : h + 1, qc * 512 : (qc + 1) * 512], in_=dstage[:]
                    )

            # ---------------- normalize OT ----------------
            nc.vector.reciprocal(recip[:], denom[:])
            nc.sync.dma_start(out=recip_d[:], in_=recip[:])
            for dt in range(8):
                div = gather.tile([128, S], F32R, tag="div")
                for half in range(2):
                    hh = dt * 2 + half
                    nc.gpsimd.dma_start(
                        out=div[half * 64 : half * 64 + 64, :],
                        in_=recip_d[hh : hh + 1, :].to_broadcast((64, S)).bitcast(F32R),
                    )
                nc.vector.tensor_mul(ot_sb[dt][:], ot_sb[dt][:], div[:])

            # ---------------- Y = OT^T @ W_OT ----------------
            for jc in range(2):
                wts = [wotp.tile([128, 512], F32R, tag=f"wot{dt}", name=f"wot{jc}_{dt}") for dt in range(8)]
                for dt in range(8):
                    nc.sync.dma_start(
                        out=wts[dt][:], in_=wot_d[dt * 128 : (dt + 1) * 128, jc * 512 : (jc + 1) * 512]
                    )
                for st in range(8):
                    py = P(st % 2)
                    for dt in range(8):
                        nc.tensor.matmul(
                            py[:], ot_sb[dt][:, st * 128 : (st + 1) * 128], wts[dt][:],
                            start=(dt == 0), stop=(dt == 7),
                        )
                    y_sb = small.tile([128, 512], F32, tag="y")
                    nc.vector.tensor_copy(y_sb[:], py[:])
                    nc.sync.dma_start(
                        out=y_d[st * 128 : (st + 1) * 128, jc * 512 : (jc + 1) * 512],
                        in_=y_sb[:],
                    )

    nc.compile()
    return nc


def host_prep(inputs):
    """Full inputs -> list of 8 per-core input maps (pure indexing/layout, no math)."""
    x = np.ascontiguousarray(np.asarray(inputs["x"], dtype=np.float32))
    cn = np.asarray(inputs["compress_neurons"], dtype=np.float32)
    ep = np.asarray(inputs["expand_pool"], dtype=np.float32)
    wot = np.ascontiguousarray(np.asarray(inputs["W_O"], dtype=np.float32).T)
    ci = np.asarray(inputs["compress_topk_idx"]).astype(np.int64)
    cw = np.asarray(inputs["compress_topk_w"], dtype=np.float32)
    maps = []
    for b in range(B):
        wvec = np.zeros((1, 64), dtype=np.float32)
        wvec[0, :KC] = cw[b]
        sele = np.empty((3 * KE, R, D), dtype=np.float32)
        for p, nm in enumerate("QKV"):
            idx = np.asarray(inputs[f"expand_topk_idx_{nm}"]).astype(np.int64)[b]
            w = np.asarray(inputs[f"expand_topk_w_{nm}"], dtype=np.float32)[b]
            sele[p * KE : (p + 1) * KE] = ep[idx]
            wvec[0, KC + p * KE : KC + (p + 1) * KE] = w
        maps.append({
            "xt": np.ascontiguousarray(x[b].T),
            "selc": np.ascontiguousarray(cn[ci[b]]),
            "sele": sele,
            "wvec": wvec,
            "wot": wot,
        })
    return maps


_COMPILED = None


def _get_compiled():
    global _COMPILED
    if _COMPILED is None:
        import jax
        from jax.sharding import Mesh, PartitionSpec
        from jax.experimental.shard_map import shard_map
        import concourse.mybir as _mybir
        from concourse.bass2jax import _bass_exec_p, install_neuronx_cc_hook, partition_id_tensor

        nc = build_program()
        install_neuronx_cc_hook()
        partition_name = nc.partition_id_tensor.name if nc.partition_id_tensor else None
        in_names, out_names, out_avals = [], [], []
        for alloc in nc.m.functions[0].allocations:
            if not isinstance(alloc, _mybir.MemoryLocationSet):
                continue
            name = alloc.memorylocations[0].name
            if alloc.kind == "ExternalInput":
                if name != partition_name:
                    in_names.append(name)
            elif alloc.kind == "ExternalOutput":
                out_names.append(name)
                out_avals.append(jax.core.ShapedArray(tuple(alloc.tensor_shape), _mybir.dt.np(alloc.dtype)))
        n_params = len(in_names)
        all_names = in_names + out_names + ([partition_name] if partition_name else [])

        def _body(*args):
            operands = list(args)
            if partition_name is not None:
                operands.append(partition_id_tensor())
            return tuple(_bass_exec_p.bind(
                *operands, out_avals=tuple(out_avals), in_names=tuple(all_names),
                out_names=tuple(out_names), lowering_input_output_aliases=(),
                sim_require_finite=False, sim_require_nnan=False, nc=nc,
            ))

        devices = jax.devices()[:B]
        mesh = Mesh(np.asarray(devices), ("core",))
        nio = n_params + len(out_names)
        fn = jax.jit(
            shard_map(_body, mesh=mesh, in_specs=(PartitionSpec("core"),) * nio,
                      out_specs=(PartitionSpec("core"),) * len(out_names), check_rep=False),
            keep_unused=True,
        )
        _COMPILED = (fn, in_names, out_names, out_avals)
    return _COMPILED


def kernel(**inputs) -> np.ndarray:
    fn, in_names, out_names, out_avals = _get_compiled()
    maps = host_prep(inputs)
    args = [np.concatenate([maps[c][n] for c in range(B)], axis=0) for n in in_names]
    args += [np.zeros((B * av.shape[0], *av.shape[1:]), av.dtype) for av in out_avals]
    import jax
    outs = fn(*args)
    jax.block_until_ready(outs)
    yi = out_names.index("y")
    y = np.asarray(outs[yi]).reshape(B, S, D).astype(np.float32)
    return y
